# revision 38
# baseline (speedup 1.0000x reference)
"""Multi-head causal attention (QKV proj + attention + out proj) on 8 TRN2
NeuronCores.

Sharding: 2-way data-parallel over batch x 4-way tensor-parallel over heads
(Megatron-style).  Core c handles batch c//4 and heads [4*(c%4), 4*(c%4)+4).
Each core computes its 4 heads' Q/K/V projections (column-parallel), the
attention for those heads, and a partial output projection (row-parallel).
The host sums the 4 TP partials per batch and adds the output bias.

v3 design notes (per core):
  - every matmul operand is fp16; PSUM accumulation stays fp32.  Host
    ships fp16 payloads (halves DMA).
  - emission interleaves proj(chunk j+1), attention(chunk j) and
    out-proj(chunk j-1) so the PE instruction stream stays dense and the
    HAM clock gate keeps the PE at 2.4 GHz.
  - attention runs on HEAD PAIRS: the QK^T contraction is only D=64, so
    the pair's score matmuls use lhsT base partitions 0/64 (disjoint PE
    row groups) and issue back-to-back -> the array runs them
    concurrently (~2x).  Scores for the pair land in one 2-bank PSUM
    tile [128, 2, 512]; one exp covers both heads (3D strided AP, zero
    dead columns); causal diagonal tiles truncate matmul/exp/mask column
    ranges to the q >= k part.
  - masks (diagonal tiles only) multiply on the otherwise-idle GPSIMD.
  - softmax denominator comes free as PSUM row D via a trailing ones
    column in V.  Normalization: ACT-copy den row to f32r -> PE
    broadcast matmul to 64 partitions -> DVE reciprocal_approx_fast ->
    one DVE multiply.  (Plain nc.vector.reciprocal is 8 cyc/elem on the
    free axis = 3.35us per row; Ln/Exp on ACT thrashes activation-table
    loads at 1.28us each.)  Only Exp/Copy are used -> one table load.
  - inputs are host-packed chunk-major ([j, p, t, s]-style) so every DMA
    lands with multi-KB contiguous per-partition runs.
"""

import numpy as np
from collections import deque
from contextlib import ExitStack

import concourse.bass as bass
import concourse.mybir as mybir
import concourse.tile as tile
from concourse import bacc
from concourse.bass import ds
from concourse.bass_utils import run_bass_kernel_spmd

B, S_FULL, E, H = 2, 2048, 1024, 16
D = E // H          # 64
NCORES = 8
TP = 4              # tensor-parallel ways (over heads)
HL = H // TP        # 4 local heads per core
F = HL * D          # 256 local projection width
P = 128
QCH = 512           # q-chunk / matmul moving-dim size
FP32 = mybir.dt.float32
F32R = mybir.dt.float32r
F16 = mybir.dt.float16
AF = mybir.ActivationFunctionType


def build(S=S_FULL, causal=True):
    ET = E // P          # 8 contraction tiles for projections
    NQ = S // QCH        # q chunks
    KT = S // P          # k tiles
    KPQ = QCH // P       # k tiles per q chunk (4)

    nc = bacc.Bacc()

    def din(name, shape, dt=F16):
        return nc.declare_dram_parameter(name, shape, dt, isOutput=False)

    # chunk-major, partition-major packed inputs (see make_in_maps)
    xq = din("xq", [NQ, P, ET, QCH])
    xk = din("xk", [NQ, P, ET, QCH])
    xv = din("xv", [NQ, P, ET, QCH])
    wq = din("wq", [P, ET, F])
    wk = din("wk", [P, ET, F])
    wv = din("wv", [P, ET, F])
    bq2 = din("bq2", [P, F // P], FP32)
    bk2 = din("bk2", [P, F // P], FP32)
    bvb = din("bvb", [P, F], FP32)
    wo = din("wo", [P, F // P, E])
    msk = din("msk", [P, KPQ, QCH], F16)
    outT = nc.declare_dram_parameter("outT", [E, S], F16, isOutput=True)

    with ExitStack() as ctx:
        ctx.enter_context(
            nc.allow_low_precision(reason="fp16 matmuls / fp16 IO are intended")
        )
        tc = ctx.enter_context(tile.TileContext(nc))
        const = ctx.enter_context(tc.tile_pool(name="const", bufs=1))
        xp = ctx.enter_context(tc.tile_pool(name="xp", bufs=4))
        pex = ctx.enter_context(tc.tile_pool(name="pex", bufs=4))
        prn = ctx.enter_context(tc.tile_pool(name="prn", bufs=4))
        opool = ctx.enter_context(tc.tile_pool(name="op", bufs=3))
        # PSUM: psc 2 bufs x 2 banks + po 2 x 1 + pp 2 x 1 = 8 banks
        psc = ctx.enter_context(tc.tile_pool(name="psc", bufs=2, space="PSUM"))
        po = ctx.enter_context(tc.tile_pool(name="po", bufs=2, space="PSUM"))
        pp = ctx.enter_context(tc.tile_pool(name="pp", bufs=2, space="PSUM"))

        # ---- constants / persistent tensors ----
        # PE clock warm-up first, fed from a memset tile (no DMA
        # dependency): back-to-back dummy matmuls (WAW-serialized on one
        # PSUM tile) keep the tensor engine busy through the HAM window
        # while the input DMAs stream, so real work starts at 2.4 GHz.
        warm_f32 = const.tile([P, QCH], FP32)
        nc.vector.memset(warm_f32, 0.125)
        warm_sb = const.tile([P, QCH], F16)
        nc.vector.tensor_copy(warm_sb, warm_f32)
        wps = pp.tile([P, QCH], FP32, tag="acc")
        for _ in range(18):
            nc.tensor.matmul(
                wps, warm_sb[:, 0:P], warm_sb, start=True, stop=True
            )

        def emit_x_dma(j, only=None):
            """Prefetch chunk j of xq/xk/xv; returns name->tile."""
            tiles = {}
            for name, src in (("q", xq), ("k", xk), ("v", xv)):
                if only and name not in only:
                    continue
                xt = xp.tile([P, ET, QCH], F16, tag="xt", name=f"xt{name}{j}")
                nc.sync.dma_start(out=xt, in_=src[j])
                tiles[name] = xt
            return tiles

        # DMA issue order = first-use order (transfers drain the shared
        # engine pool roughly in issue order, and each dma_start costs
        # ~0.6us of Sync issue time): Q weights + chunk-0 xq gate the
        # first projection matmuls; masks/wo are needed much later.
        wq_sb = const.tile([P, ET, F], F16)
        nc.sync.dma_start(out=wq_sb, in_=wq[:, :, :])
        bq_sb = const.tile([P, F // P], FP32)
        nc.sync.dma_start(out=bq_sb, in_=bq2[:, :])
        xtiles0 = emit_x_dma(0, only="q")
        wk_sb = const.tile([P, ET, F], F16)
        nc.sync.dma_start(out=wk_sb, in_=wk[:, :, :])
        bk_sb = const.tile([P, F // P], FP32)
        nc.sync.dma_start(out=bk_sb, in_=bk2[:, :])
        xtiles0.update(emit_x_dma(0, only="k"))
        wv_sb = const.tile([P, ET, F], F16)
        nc.sync.dma_start(out=wv_sb, in_=wv[:, :, :])
        bvb_sb = const.tile([P, F], FP32)
        nc.sync.dma_start(out=bvb_sb, in_=bvb[:, :])
        xtiles0.update(emit_x_dma(0, only="v"))
        msk_sb = const.tile([P, KPQ, QCH], F16)
        nc.sync.dma_start(out=msk_sb, in_=msk[:, :, :])
        xtiles_next = emit_x_dma(1) if NQ > 1 else None
        wo_sb = const.tile([P, F // P, E], F16)
        nc.sync.dma_start(out=wo_sb, in_=wo[:, :, :])
        ones_f32 = const.tile([P, D], FP32)
        nc.vector.memset(ones_f32, 1.0)

        qT_sb = const.tile([P, F // P, S], F16)
        kT_sb = const.tile([P, F // P, S], F16)
        # V with a trailing ones column: AV matmul emits the softmax
        # denominator as PSUM row D for free.
        vo_sb = const.tile([P, KT, HL, D + 1], F16)
        nc.scalar.activation(
            vo_sb[:, :, :, D:D + 1],
            ones_f32[:, 0:KT * HL].rearrange("p (a b c) -> p a b c", a=KT, b=HL, c=1),
            AF.Copy,
        )
        oT_sb = const.tile([P, F // P, S], F16)

        # ---- emission helpers ----
        def proj_groups(j, xtiles):
            """8 closures: Q blk0/1, K blk0/1, V sl0..3 for chunk j."""
            groups = []

            def qk_group(xt, w_sb, b_sb, dst, blk):
                def emit():
                    acc = pp.tile([P, QCH], FP32, tag="acc")
                    for et in range(ET):
                        nc.tensor.matmul(
                            acc,
                            w_sb[:, et, ds(blk * P, P)],
                            xt[:, et, :],
                            start=(et == 0),
                            stop=(et == ET - 1),
                        )
                    nc.vector.tensor_scalar_add(
                        dst[:, blk, ds(j * QCH, QCH)], acc, b_sb[:, blk:blk + 1]
                    )
                return emit

            def v_group(xt, sl):
                def emit():
                    st = j * (QCH // P) + sl
                    acc = pp.tile([P, QCH], FP32, tag="acc")
                    for et in range(ET):
                        nc.tensor.matmul(
                            acc[:, 0:F],
                            xt[:, et, ds(sl * P, P)],
                            wv_sb[:, et, :],
                            start=(et == 0),
                            stop=(et == ET - 1),
                        )
                    for h in range(HL):
                        nc.vector.tensor_add(
                            vo_sb[:, st, h, 0:D],
                            acc[:, ds(h * D, D)],
                            bvb_sb[:, ds(h * D, D)],
                        )
                return emit

            for blk in range(F // P):
                groups.append(qk_group(xtiles["q"], wq_sb, bq_sb, qT_sb, blk))
            for blk in range(F // P):
                groups.append(qk_group(xtiles["k"], wk_sb, bk_sb, kT_sb, blk))
            for sl in range(QCH // P):
                groups.append(v_group(xtiles["v"], sl))
            return groups

        def outproj_groups(j, tail=False):
            """8 closures: one per output row-block eb for chunk j.  In the
            tail (last chunk, nothing left to overlap) the PSUM->SBUF casts
            alternate between DVE and ACT so they drain ~2x faster."""
            groups = []

            def eb_group(eb):
                def emit():
                    acc = pp.tile([P, QCH], FP32, tag="acc")
                    for fb in range(F // P):
                        nc.tensor.matmul(
                            acc,
                            wo_sb[:, fb, ds(eb * P, P)],
                            oT_sb[:, fb, ds(j * QCH, QCH)],
                            start=(fb == 0),
                            stop=(fb == F // P - 1),
                        )
                    ot = opool.tile([P, QCH], F16, tag="ot")
                    if tail and eb % 2 == 1:
                        nc.scalar.activation(ot, acc, AF.Copy)
                    else:
                        nc.vector.tensor_copy(ot, acc)
                    nc.sync.dma_start(
                        out=outT[ds(eb * P, P), ds(j * QCH, QCH)], in_=ot
                    )
                return emit

            for eb in range(E // P):
                groups.append(eb_group(eb))
            return groups

        def emit_normalize(j, hp, po_pair, tail=False):
            """oT[h, jQ] = po/den for both heads of pair hp of chunk j.
            The chain is deliberately PE-free (DVE recip on the den row,
            GPSIMD partition broadcast, DVE multiply) so the next pair's
            PV matmuls -- which reuse these PSUM banks -- never wait on a
            PE instruction emitted after them (in-order PE queue)."""
            blkh = hp
            for half in range(2):
                po_t = po_pair[half]
                doff = half * D
                rec = prn.tile([P, QCH], FP32, tag="rec")
                den = prn.tile([P, QCH], FP32, tag="den")
                nc.scalar.activation(den[0:1, :], po_t[D:D + 1, :], AF.Copy)
                rr = prn.tile([P, QCH], FP32, tag="rr")
                nc.vector.reciprocal_approx_fast(rr[0:1, :], den[0:1, :])
                nc.gpsimd.partition_broadcast(rec[0:D, :], rr[0:1, :])
                nc.vector.tensor_mul(
                    oT_sb[doff:doff + D, blkh, ds(j * QCH, QCH)],
                    po_t[0:D, :],
                    rec[0:D, :],
                )

        def attention_pair(j, hp, filler, stride=2):
            """Scores+exp+mask+PV for head pair hp (heads 2hp, 2hp+1) of
            chunk j.  The two heads' score matmuls are issued back-to-back
            with lhsT base partitions 0/64 -> concurrent PE row groups.
            Pops filler closures between k-tiles to keep PE dense."""
            blkh = hp
            nkt = KPQ * (j + 1) if causal else KT

            def trunc(kt):
                if causal and kt >= KPQ * j:
                    return (kt - KPQ * j) * P
                return 0

            po_pair = (po.tile([P, QCH], FP32, tag="po", name="poA"),
                       po.tile([P, QCH], FP32, tag="po", name="poB"))
            for kt in range(nkt):
                t0 = trunc(kt)
                w = QCH - t0
                sc = psc.tile([P, 2, QCH], FP32, tag="sc")
                for half in range(2):
                    doff = half * D
                    nc.tensor.matmul(
                        sc[:, half, ds(t0, w)],
                        kT_sb[doff:doff + D, blkh, ds(kt * P, P)],
                        qT_sb[doff:doff + D, blkh, ds(j * QCH + t0, w)],
                        start=True,
                        stop=True,
                    )
                pt = pex.tile([P, 2, QCH], F16, tag="pt")
                nc.scalar.activation(
                    pt[:, :, ds(t0, w)], sc[:, :, ds(t0, w)], AF.Exp
                )
                if causal and kt >= KPQ * j:
                    # only the first 128 columns of a diagonal tile contain
                    # the partial triangle; every column q >= t0+128 already
                    # has q > k for all k rows of this tile -> no mask.
                    t = kt - KPQ * j
                    for half in range(2):
                        nc.vector.tensor_mul(
                            pt[:, half, ds(t0, P)],
                            pt[:, half, ds(t0, P)],
                            msk_sb[:, t, ds(t0, P)],
                        )
                for half in range(2):
                    h = 2 * hp + half
                    nc.tensor.matmul(
                        po_pair[half][0:D + 1, ds(t0, w)],
                        vo_sb[:, kt, h, :],
                        pt[:, half, ds(t0, w)],
                        start=(kt == 0),
                        stop=(kt == nkt - 1),
                    )
                if filler and kt % stride == stride - 1:
                    filler.popleft()()
            return po_pair

        # ---- pipelined main loop ----
        for g in proj_groups(0, xtiles0):
            g()

        reserve = deque()
        for j in range(NQ):
            filler = deque()
            if j + 1 < NQ:
                filler.extend(proj_groups(j + 1, xtiles_next))
                if j + 2 < NQ:
                    xtiles_next = emit_x_dma(j + 2)
            if j >= 1:
                op_groups = outproj_groups(j - 1)
                if j == NQ - 1:
                    # hold 4 groups back: they fill the PE while the last
                    # pair's normalization chain drains on ACT/DVE/GPSIMD.
                    reserve.extend(op_groups[4:])
                    op_groups = op_groups[:4]
                filler.extend(op_groups)
            for hp in range(HL // 2):
                po_pair = attention_pair(j, hp, filler)
                emit_normalize(j, hp, po_pair)
            while filler:
                filler.popleft()()

        while reserve:
            reserve.popleft()()
        for g in outproj_groups(NQ - 1, tail=True):
            g()

    nc.compile()
    return nc


def make_masks(S=S_FULL):
    KPQ = QCH // P
    m = np.zeros((P, KPQ, QCH), np.float32)
    for t in range(KPQ):
        kk = np.arange(P)[:, None]
        qq = np.arange(QCH)[None, :]
        m[:, t, :] = (qq >= kk + P * t).astype(np.float32)
    return m


def _pack_x(xT, S):
    """[E, S] fp -> chunk-major [NQ, P, ET, QCH] fp16 (contiguous runs)."""
    NQ, ET = S // QCH, E // P
    x = xT.reshape(ET, P, NQ, QCH).transpose(2, 1, 0, 3)
    return np.ascontiguousarray(x.astype(np.float16))


def _pack_w(wT, width):
    """[E, width] -> [P, ET, width] fp16."""
    ET = E // P
    w = wT.reshape(ET, P, width).transpose(1, 0, 2)
    return np.ascontiguousarray(w.astype(np.float16))


def make_in_maps(query, key, value, Wq, bq, Wk, bk, Wv, bv, Wo, bo, S=S_FULL):
    scale = float(D) ** -0.5
    q = np.asarray(query, np.float32)
    k = np.asarray(key, np.float32)
    v = np.asarray(value, np.float32)
    Wq = np.asarray(Wq, np.float32)
    Wk = np.asarray(Wk, np.float32)
    Wv = np.asarray(Wv, np.float32)
    Wo = np.asarray(Wo, np.float32)
    bq = np.asarray(bq, np.float32)
    bk = np.asarray(bk, np.float32)
    bv = np.asarray(bv, np.float32)
    masks = make_masks(S)
    in_maps = []
    for c in range(NCORES):
        b, tp = divmod(c, TP)
        rows = slice(tp * F, (tp + 1) * F)
        wo_p = Wo[:, rows].T.reshape(F // P, P, E).transpose(1, 0, 2)
        in_maps.append({
            "xq": _pack_x(q[b].T, S),
            "xk": _pack_x(k[b].T, S),
            "xv": _pack_x(v[b].T, S),
            "wq": _pack_w((Wq[rows] * scale).T, F),
            "wk": _pack_w(Wk[rows].T, F),
            "wv": _pack_w(Wv[rows].T, F),
            "bq2": np.ascontiguousarray((bq[rows] * scale).reshape(F // P, P).T),
            "bk2": np.ascontiguousarray(bk[rows].reshape(F // P, P).T),
            "bvb": np.ascontiguousarray(np.broadcast_to(bv[rows], (P, F))),
            "wo": np.ascontiguousarray(wo_p.astype(np.float16)),
            "msk": masks.astype(np.float16),
        })
    return in_maps


_CACHE = {}


def _get_nc(causal):
    if causal not in _CACHE:
        _CACHE[causal] = build(S_FULL, causal)
    return _CACHE[causal]


def kernel(query, key, value, Wq, bq, Wk, bk, Wv, bv, Wo, bo, is_causal):
    causal = bool(int(np.asarray(is_causal)))
    nc = _get_nc(causal)
    in_maps = make_in_maps(query, key, value, Wq, bq, Wk, bk, Wv, bv, Wo, bo)
    res = run_bass_kernel_spmd(nc, in_maps, core_ids=list(range(NCORES)))
    out = np.zeros((B, S_FULL, E), np.float32)
    for c in range(NCORES):
        b, tp = divmod(c, TP)
        out[b] += res.results[c]["outT"].astype(np.float32).T
    out += np.asarray(bo, np.float32)
    return out


# revision 39
# speedup vs baseline: 1.1579x; 1.1579x over previous
"""Multi-head causal attention (QKV proj + attention + out proj) on 8 TRN2
NeuronCores.

Sharding: 2-way data-parallel over batch x 4-way tensor-parallel over heads
(Megatron-style).  Core c handles batch c//4 and heads [4*(c%4), 4*(c%4)+4).
Each core computes its 4 heads' Q/K/V projections (column-parallel), the
attention for those heads, and a partial output projection (row-parallel).
The host sums the 4 TP partials per batch and adds the output bias.

v3 design notes (per core):
  - every matmul operand is fp16; PSUM accumulation stays fp32.  Host
    ships fp16 payloads (halves DMA).
  - emission interleaves proj(chunk j+1), attention(chunk j) and
    out-proj(chunk j-1) so the PE instruction stream stays dense and the
    HAM clock gate keeps the PE at 2.4 GHz.
  - attention runs on HEAD PAIRS: the QK^T contraction is only D=64, so
    the pair's score matmuls use lhsT base partitions 0/64 (disjoint PE
    row groups) and issue back-to-back -> the array runs them
    concurrently (~2x).  Scores for the pair land in one 2-bank PSUM
    tile [128, 2, 512]; one exp covers both heads (3D strided AP, zero
    dead columns); causal diagonal tiles truncate matmul/exp/mask column
    ranges to the q >= k part.
  - masks (diagonal tiles only) multiply on the otherwise-idle GPSIMD.
  - softmax denominator comes free as PSUM row D via a trailing ones
    column in V.  Normalization: ACT-copy den row to f32r -> PE
    broadcast matmul to 64 partitions -> DVE reciprocal_approx_fast ->
    one DVE multiply.  (Plain nc.vector.reciprocal is 8 cyc/elem on the
    free axis = 3.35us per row; Ln/Exp on ACT thrashes activation-table
    loads at 1.28us each.)  Only Exp/Copy are used -> one table load.
  - inputs are host-packed chunk-major ([j, p, t, s]-style) so every DMA
    lands with multi-KB contiguous per-partition runs.
"""

import numpy as np
from collections import deque
from contextlib import ExitStack

import concourse.bass as bass
import concourse.mybir as mybir
import concourse.tile as tile
from concourse import bacc
from concourse.bass import ds
from concourse.bass_utils import run_bass_kernel_spmd

B, S_FULL, E, H = 2, 2048, 1024, 16
D = E // H          # 64
NCORES = 8
TP = 4              # tensor-parallel ways (over heads)
HL = H // TP        # 4 local heads per core
F = HL * D          # 256 local projection width
P = 128
QCH = 512           # q-chunk / matmul moving-dim size
FP32 = mybir.dt.float32
F32R = mybir.dt.float32r
F16 = mybir.dt.float16
AF = mybir.ActivationFunctionType


def build(S=S_FULL, causal=True):
    ET = E // P          # 8 contraction tiles for projections
    NQ = S // QCH        # q chunks
    KT = S // P          # k tiles
    KPQ = QCH // P       # k tiles per q chunk (4)

    nc = bacc.Bacc()

    def din(name, shape, dt=F16):
        return nc.declare_dram_parameter(name, shape, dt, isOutput=False)

    # chunk-major, partition-major packed inputs (see make_in_maps)
    xq = din("xq", [NQ, P, ET, QCH])
    xk = din("xk", [NQ, P, ET, QCH])
    xv = din("xv", [NQ, P, ET, QCH])
    wq = din("wq", [P, ET, F])
    wk = din("wk", [P, ET, F])
    wv = din("wv", [P, ET, F])
    bq2 = din("bq2", [P, F // P], FP32)
    bk2 = din("bk2", [P, F // P], FP32)
    bvb = din("bvb", [P, F], FP32)
    wo = din("wo", [P, F // P, E])
    msk = din("msk", [P, KPQ, QCH], F16)
    outT = nc.declare_dram_parameter("outT", [E, S], F16, isOutput=True)

    with ExitStack() as ctx:
        ctx.enter_context(
            nc.allow_low_precision(reason="fp16 matmuls / fp16 IO are intended")
        )
        tc = ctx.enter_context(tile.TileContext(nc))
        const = ctx.enter_context(tc.tile_pool(name="const", bufs=1))
        xp = ctx.enter_context(tc.tile_pool(name="xp", bufs=4))
        pex = ctx.enter_context(tc.tile_pool(name="pex", bufs=4))
        prn = ctx.enter_context(tc.tile_pool(name="prn", bufs=4))
        opool = ctx.enter_context(tc.tile_pool(name="op", bufs=3))
        # PSUM: psc 2 bufs x 2 banks + po 2 x 1 + pp 2 x 1 = 8 banks
        psc = ctx.enter_context(tc.tile_pool(name="psc", bufs=2, space="PSUM"))
        po = ctx.enter_context(tc.tile_pool(name="po", bufs=2, space="PSUM"))
        pp = ctx.enter_context(tc.tile_pool(name="pp", bufs=2, space="PSUM"))

        # ---- constants / persistent tensors ----
        # PE clock warm-up first, fed from a memset tile (no DMA
        # dependency): back-to-back dummy matmuls (WAW-serialized on one
        # PSUM tile) keep the tensor engine busy through the HAM window
        # while the input DMAs stream, so real work starts at 2.4 GHz.
        warm_f32 = const.tile([P, QCH], FP32)
        nc.vector.memset(warm_f32, 0.125)
        warm_sb = const.tile([P, QCH], F16)
        nc.vector.tensor_copy(warm_sb, warm_f32)
        wps = pp.tile([P, QCH], FP32, tag="acc")
        for _ in range(18):
            nc.tensor.matmul(
                wps, warm_sb[:, 0:P], warm_sb, start=True, stop=True
            )

        def emit_x_dma(j, only=None):
            """Prefetch chunk j of xq/xk/xv; returns name->tile."""
            tiles = {}
            for name, src in (("q", xq), ("k", xk), ("v", xv)):
                if only and name not in only:
                    continue
                xt = xp.tile([P, ET, QCH], F16, tag="xt", name=f"xt{name}{j}")
                nc.sync.dma_start(out=xt, in_=src[j])
                tiles[name] = xt
            return tiles

        # DMA issue order = first-use order (transfers drain the shared
        # engine pool roughly in issue order, and each dma_start costs
        # ~0.6us of Sync issue time): Q weights + chunk-0 xq gate the
        # first projection matmuls; masks/wo are needed much later.
        wq_sb = const.tile([P, ET, F], F16)
        nc.sync.dma_start(out=wq_sb, in_=wq[:, :, :])
        bq_sb = const.tile([P, F // P], FP32)
        nc.sync.dma_start(out=bq_sb, in_=bq2[:, :])
        xtiles0 = emit_x_dma(0, only="q")
        wk_sb = const.tile([P, ET, F], F16)
        nc.sync.dma_start(out=wk_sb, in_=wk[:, :, :])
        bk_sb = const.tile([P, F // P], FP32)
        nc.sync.dma_start(out=bk_sb, in_=bk2[:, :])
        xtiles0.update(emit_x_dma(0, only="k"))
        wv_sb = const.tile([P, ET, F], F16)
        nc.sync.dma_start(out=wv_sb, in_=wv[:, :, :])
        bvb_sb = const.tile([P, F], FP32)
        nc.sync.dma_start(out=bvb_sb, in_=bvb[:, :])
        xtiles0.update(emit_x_dma(0, only="v"))
        msk_sb = const.tile([P, KPQ, QCH], F16)
        nc.sync.dma_start(out=msk_sb, in_=msk[:, :, :])
        xtiles_next = emit_x_dma(1) if NQ > 1 else None
        wo_sb = const.tile([P, F // P, E], F16)
        nc.sync.dma_start(out=wo_sb, in_=wo[:, :, :])
        ones_f32 = const.tile([P, D], FP32)
        nc.vector.memset(ones_f32, 1.0)

        qT_sb = const.tile([P, F // P, S], F16)
        kT_sb = const.tile([P, F // P, S], F16)
        # V with a trailing ones column: AV matmul emits the softmax
        # denominator as PSUM row D for free.
        vo_sb = const.tile([P, KT, HL, D + 1], F16)
        nc.scalar.activation(
            vo_sb[:, :, :, D:D + 1],
            ones_f32[:, 0:KT * HL].rearrange("p (a b c) -> p a b c", a=KT, b=HL, c=1),
            AF.Copy,
        )
        oT_sb = const.tile([P, F // P, S], F16)

        # ---- emission helpers ----
        def proj_groups(j, xtiles):
            """8 closures: Q blk0/1, K blk0/1, V sl0..3 for chunk j."""
            groups = []

            def qk_group(xt, w_sb, b_sb, dst, blk):
                def emit():
                    acc = pp.tile([P, QCH], FP32, tag="acc")
                    for et in range(ET):
                        nc.tensor.matmul(
                            acc,
                            w_sb[:, et, ds(blk * P, P)],
                            xt[:, et, :],
                            start=(et == 0),
                            stop=(et == ET - 1),
                        )
                    nc.vector.tensor_scalar_add(
                        dst[:, blk, ds(j * QCH, QCH)], acc, b_sb[:, blk:blk + 1]
                    )
                return emit

            def v_group(xt, sl):
                def emit():
                    st = j * (QCH // P) + sl
                    acc = pp.tile([P, QCH], FP32, tag="acc")
                    for et in range(ET):
                        nc.tensor.matmul(
                            acc[:, 0:F],
                            xt[:, et, ds(sl * P, P)],
                            wv_sb[:, et, :],
                            start=(et == 0),
                            stop=(et == ET - 1),
                        )
                    for h in range(HL):
                        nc.vector.tensor_add(
                            vo_sb[:, st, h, 0:D],
                            acc[:, ds(h * D, D)],
                            bvb_sb[:, ds(h * D, D)],
                        )
                return emit

            for blk in range(F // P):
                groups.append(qk_group(xtiles["q"], wq_sb, bq_sb, qT_sb, blk))
            for blk in range(F // P):
                groups.append(qk_group(xtiles["k"], wk_sb, bk_sb, kT_sb, blk))
            for sl in range(QCH // P):
                groups.append(v_group(xtiles["v"], sl))
            return groups

        def outproj_groups(j, tail=False):
            """8 closures: one per output row-block eb for chunk j.  In the
            tail (last chunk, nothing left to overlap) the PSUM->SBUF casts
            alternate between DVE and ACT so they drain ~2x faster."""
            groups = []

            def eb_group(eb):
                def emit():
                    acc = pp.tile([P, QCH], FP32, tag="acc")
                    for fb in range(F // P):
                        nc.tensor.matmul(
                            acc,
                            wo_sb[:, fb, ds(eb * P, P)],
                            oT_sb[:, fb, ds(j * QCH, QCH)],
                            start=(fb == 0),
                            stop=(fb == F // P - 1),
                        )
                    ot = opool.tile([P, QCH], F16, tag="ot")
                    if tail and eb % 2 == 1:
                        nc.scalar.activation(ot, acc, AF.Copy)
                    else:
                        nc.vector.tensor_copy(ot, acc)
                    nc.sync.dma_start(
                        out=outT[ds(eb * P, P), ds(j * QCH, QCH)], in_=ot
                    )
                return emit

            for eb in range(E // P):
                groups.append(eb_group(eb))
            return groups

        def emit_normalize(j, hp, po_pair, tail=False):
            """oT[h, jQ] = po/den for both heads of pair hp of chunk j.
            The chain is deliberately PE-free (DVE recip on the den row,
            GPSIMD partition broadcast, DVE multiply) so the next pair's
            PV matmuls -- which reuse these PSUM banks -- never wait on a
            PE instruction emitted after them (in-order PE queue)."""
            blkh = hp
            for half in range(2):
                po_t = po_pair[half]
                doff = half * D
                rec = prn.tile([P, QCH], FP32, tag="rec")
                den = prn.tile([P, QCH], FP32, tag="den")
                nc.scalar.activation(den[0:1, :], po_t[D:D + 1, :], AF.Copy)
                rr = prn.tile([P, QCH], FP32, tag="rr")
                nc.vector.reciprocal_approx_fast(rr[0:1, :], den[0:1, :])
                nc.gpsimd.partition_broadcast(rec[0:D, :], rr[0:1, :])
                nc.vector.tensor_mul(
                    oT_sb[doff:doff + D, blkh, ds(j * QCH, QCH)],
                    po_t[0:D, :],
                    rec[0:D, :],
                )

        def attention_pair(j, hp, filler, stride=2):
            """Scores+exp+mask+PV for head pair hp (heads 2hp, 2hp+1) of
            chunk j.  The two heads' score matmuls are issued back-to-back
            with lhsT base partitions 0/64 -> concurrent PE row groups.
            Pops filler closures between k-tiles to keep PE dense."""
            blkh = hp
            nkt = KPQ * (j + 1) if causal else KT

            def trunc(kt):
                if causal and kt >= KPQ * j:
                    return (kt - KPQ * j) * P
                return 0

            po_pair = (po.tile([P, QCH], FP32, tag="po", name="poA"),
                       po.tile([P, QCH], FP32, tag="po", name="poB"))
            for kt in range(nkt):
                t0 = trunc(kt)
                w = QCH - t0
                sc = psc.tile([P, 2, QCH], FP32, tag="sc")
                for half in range(2):
                    doff = half * D
                    nc.tensor.matmul(
                        sc[:, half, ds(t0, w)],
                        kT_sb[doff:doff + D, blkh, ds(kt * P, P)],
                        qT_sb[doff:doff + D, blkh, ds(j * QCH + t0, w)],
                        start=True,
                        stop=True,
                    )
                pt = pex.tile([P, 2, QCH], F16, tag="pt")
                nc.scalar.activation(
                    pt[:, :, ds(t0, w)], sc[:, :, ds(t0, w)], AF.Exp
                )
                if causal and kt >= KPQ * j:
                    # NOTE: masking the full [t0, 512) range (not just the
                    # 128-col partial triangle) keeps the mask op the sole
                    # last-writer of the PV read range -- narrowing it to
                    # 128 cols measured 27us SLOWER (extra PE-queue
                    # semaphore wait per diagonal PV).
                    t = kt - KPQ * j
                    for half in range(2):
                        nc.vector.tensor_mul(
                            pt[:, half, ds(t0, w)],
                            pt[:, half, ds(t0, w)],
                            msk_sb[:, t, ds(t0, w)],
                        )
                for half in range(2):
                    h = 2 * hp + half
                    nc.tensor.matmul(
                        po_pair[half][0:D + 1, ds(t0, w)],
                        vo_sb[:, kt, h, :],
                        pt[:, half, ds(t0, w)],
                        start=(kt == 0),
                        stop=(kt == nkt - 1),
                    )
                if filler and kt % stride == stride - 1:
                    filler.popleft()()
            return po_pair

        # ---- pipelined main loop ----
        for g in proj_groups(0, xtiles0):
            g()

        reserve = deque()
        for j in range(NQ):
            filler = deque()
            if j + 1 < NQ:
                filler.extend(proj_groups(j + 1, xtiles_next))
                if j + 2 < NQ:
                    xtiles_next = emit_x_dma(j + 2)
            if j >= 1:
                op_groups = outproj_groups(j - 1)
                if j == NQ - 1:
                    # hold 4 groups back: they fill the PE while the last
                    # pair's normalization chain drains on ACT/DVE/GPSIMD.
                    reserve.extend(op_groups[4:])
                    op_groups = op_groups[:4]
                filler.extend(op_groups)
            for hp in range(HL // 2):
                po_pair = attention_pair(j, hp, filler)
                emit_normalize(j, hp, po_pair)
            while filler:
                filler.popleft()()

        while reserve:
            reserve.popleft()()
        for g in outproj_groups(NQ - 1, tail=True):
            g()

    nc.compile()
    return nc


def make_masks(S=S_FULL):
    KPQ = QCH // P
    m = np.zeros((P, KPQ, QCH), np.float32)
    for t in range(KPQ):
        kk = np.arange(P)[:, None]
        qq = np.arange(QCH)[None, :]
        m[:, t, :] = (qq >= kk + P * t).astype(np.float32)
    return m


def _pack_x(xT, S):
    """[E, S] fp -> chunk-major [NQ, P, ET, QCH] fp16 (contiguous runs)."""
    NQ, ET = S // QCH, E // P
    x = xT.reshape(ET, P, NQ, QCH).transpose(2, 1, 0, 3)
    return np.ascontiguousarray(x.astype(np.float16))


def _pack_w(wT, width):
    """[E, width] -> [P, ET, width] fp16."""
    ET = E // P
    w = wT.reshape(ET, P, width).transpose(1, 0, 2)
    return np.ascontiguousarray(w.astype(np.float16))


def make_in_maps(query, key, value, Wq, bq, Wk, bk, Wv, bv, Wo, bo, S=S_FULL):
    scale = float(D) ** -0.5
    q = np.asarray(query, np.float32)
    k = np.asarray(key, np.float32)
    v = np.asarray(value, np.float32)
    Wq = np.asarray(Wq, np.float32)
    Wk = np.asarray(Wk, np.float32)
    Wv = np.asarray(Wv, np.float32)
    Wo = np.asarray(Wo, np.float32)
    bq = np.asarray(bq, np.float32)
    bk = np.asarray(bk, np.float32)
    bv = np.asarray(bv, np.float32)
    masks = make_masks(S)
    in_maps = []
    for c in range(NCORES):
        b, tp = divmod(c, TP)
        rows = slice(tp * F, (tp + 1) * F)
        wo_p = Wo[:, rows].T.reshape(F // P, P, E).transpose(1, 0, 2)
        in_maps.append({
            "xq": _pack_x(q[b].T, S),
            "xk": _pack_x(k[b].T, S),
            "xv": _pack_x(v[b].T, S),
            "wq": _pack_w((Wq[rows] * scale).T, F),
            "wk": _pack_w(Wk[rows].T, F),
            "wv": _pack_w(Wv[rows].T, F),
            "bq2": np.ascontiguousarray((bq[rows] * scale).reshape(F // P, P).T),
            "bk2": np.ascontiguousarray(bk[rows].reshape(F // P, P).T),
            "bvb": np.ascontiguousarray(np.broadcast_to(bv[rows], (P, F))),
            "wo": np.ascontiguousarray(wo_p.astype(np.float16)),
            "msk": masks.astype(np.float16),
        })
    return in_maps


_CACHE = {}


def _get_nc(causal):
    if causal not in _CACHE:
        _CACHE[causal] = build(S_FULL, causal)
    return _CACHE[causal]


def kernel(query, key, value, Wq, bq, Wk, bk, Wv, bv, Wo, bo, is_causal):
    causal = bool(int(np.asarray(is_causal)))
    nc = _get_nc(causal)
    in_maps = make_in_maps(query, key, value, Wq, bq, Wk, bk, Wv, bv, Wo, bo)
    res = run_bass_kernel_spmd(nc, in_maps, core_ids=list(range(NCORES)))
    out = np.zeros((B, S_FULL, E), np.float32)
    for c in range(NCORES):
        b, tp = divmod(c, TP)
        out[b] += res.results[c]["outT"].astype(np.float32).T
    out += np.asarray(bo, np.float32)
    return out


# revision 40
# speedup vs baseline: 1.1620x; 1.0036x over previous
"""Multi-head causal attention (QKV proj + attention + out proj) on 8 TRN2
NeuronCores.

Sharding: 2-way data-parallel over batch x 4-way tensor-parallel over heads
(Megatron-style).  Core c handles batch c//4 and heads [4*(c%4), 4*(c%4)+4).
Each core computes its 4 heads' Q/K/V projections (column-parallel), the
attention for those heads, and a partial output projection (row-parallel).
The host sums the 4 TP partials per batch and adds the output bias.

Design notes (per core), measured 329us (fp32r baseline) -> ~168us:
  - every matmul operand is fp16; PSUM accumulation stays fp32.  Host
    ships fp16 payloads (halves DMA; 16-bit weights/activations are well
    inside the 2e-2 rel-err budget, measured 4.5e-4).
  - emission interleaves proj(chunk j+1), attention(chunk j) and
    out-proj(chunk j-1) closures so the PE instruction stream stays
    dense and the HAM clock gate keeps the PE at 2.4 GHz (the phase-
    sequential baseline spent 157us throttled at 1.2 GHz).
  - attention runs on HEAD PAIRS: the QK^T contraction is only D=64, so
    the pair's score matmuls use lhsT base partitions 0/64 (disjoint PE
    row groups) and issue back-to-back -> the array runs them
    concurrently (measured dstart ~3ns).  Scores for the pair land in
    one 2-bank PSUM tile [128, 2, 512]; one exp covers both heads (3D
    strided AP, zero dead columns); causal diagonal tiles truncate
    matmul/exp/mask column ranges to the q >= k part.
  - diagonal masks multiply the full truncated range on DVE: narrowing
    them to the 128-col partial triangle measured SLOWER (the mask op
    stops being the sole last-writer of the PV read range, adding a
    PE-queue semaphore wait per diagonal PV).
  - softmax denominator comes free as PSUM row D via a trailing ones
    column in V.  Normalization per head: ACT-copy den row to SBUF ->
    DVE reciprocal_approx_fast on the row -> GPSIMD partition_broadcast
    to 64 partitions -> one DVE multiply.  This chain is deliberately
    PE-free (no in-order-queue deadlock/stall against the next pair's
    PV matmuls reusing the po banks) and avoids both the 3.35us
    nc.vector.reciprocal (8 cyc per free-dim element regardless of
    partition count) and Ln/Exp activation-table thrashing (1.28us per
    ACT_TABLE_LOAD; exp and ln live in different default table sets).
    reciprocal_approx_fast needs an SBUF fp32 input -- feeding it PSUM
    directly returns garbage.
  - inputs are host-packed chunk-major ([j, p, t, s]-style) so every DMA
    lands with multi-KB contiguous per-partition runs; DMA issue order
    is first-use order (wq/xq0 before everything else).
  - PSUM budget exactly 8 banks: psc 2x2 (score pairs) + po 2x1 (PV
    accumulators) + pp 2x1 (proj/out-proj accumulators + warmup).
"""

import numpy as np
from collections import deque
from contextlib import ExitStack

import concourse.bass as bass
import concourse.mybir as mybir
import concourse.tile as tile
from concourse import bacc
from concourse.bass import ds
from concourse.bass_utils import run_bass_kernel_spmd

B, S_FULL, E, H = 2, 2048, 1024, 16
D = E // H          # 64
NCORES = 8
TP = 4              # tensor-parallel ways (over heads)
HL = H // TP        # 4 local heads per core
F = HL * D          # 256 local projection width
P = 128
QCH = 512           # q-chunk / matmul moving-dim size
FP32 = mybir.dt.float32
F32R = mybir.dt.float32r
F16 = mybir.dt.float16
AF = mybir.ActivationFunctionType


def build(S=S_FULL, causal=True):
    ET = E // P          # 8 contraction tiles for projections
    NQ = S // QCH        # q chunks
    KT = S // P          # k tiles
    KPQ = QCH // P       # k tiles per q chunk (4)

    nc = bacc.Bacc()

    def din(name, shape, dt=F16):
        return nc.declare_dram_parameter(name, shape, dt, isOutput=False)

    # chunk-major, partition-major packed inputs (see make_in_maps)
    xq = din("xq", [NQ, P, ET, QCH])
    xk = din("xk", [NQ, P, ET, QCH])
    xv = din("xv", [NQ, P, ET, QCH])
    wq = din("wq", [P, ET, F])
    wk = din("wk", [P, ET, F])
    wv = din("wv", [P, ET, F])
    bq2 = din("bq2", [P, F // P], FP32)
    bk2 = din("bk2", [P, F // P], FP32)
    bvb = din("bvb", [P, F], FP32)
    wo = din("wo", [P, F // P, E])
    msk = din("msk", [P, KPQ, QCH], F16)
    outT = nc.declare_dram_parameter("outT", [E, S], F16, isOutput=True)

    with ExitStack() as ctx:
        ctx.enter_context(
            nc.allow_low_precision(reason="fp16 matmuls / fp16 IO are intended")
        )
        tc = ctx.enter_context(tile.TileContext(nc))
        const = ctx.enter_context(tc.tile_pool(name="const", bufs=1))
        xp = ctx.enter_context(tc.tile_pool(name="xp", bufs=4))
        pex = ctx.enter_context(tc.tile_pool(name="pex", bufs=4))
        prn = ctx.enter_context(tc.tile_pool(name="prn", bufs=4))
        opool = ctx.enter_context(tc.tile_pool(name="op", bufs=3))
        # PSUM: psc 2 bufs x 2 banks + po 2 x 1 + pp 2 x 1 = 8 banks
        psc = ctx.enter_context(tc.tile_pool(name="psc", bufs=2, space="PSUM"))
        po = ctx.enter_context(tc.tile_pool(name="po", bufs=2, space="PSUM"))
        pp = ctx.enter_context(tc.tile_pool(name="pp", bufs=2, space="PSUM"))

        # ---- constants / persistent tensors ----
        # PE clock warm-up first, fed from a memset tile (no DMA
        # dependency): back-to-back dummy matmuls (WAW-serialized on one
        # PSUM tile) keep the tensor engine busy through the HAM window
        # while the input DMAs stream, so real work starts at 2.4 GHz.
        warm_f32 = const.tile([P, QCH], FP32)
        nc.vector.memset(warm_f32, 0.125)
        warm_sb = const.tile([P, QCH], F16)
        nc.vector.tensor_copy(warm_sb, warm_f32)
        wps = pp.tile([P, QCH], FP32, tag="acc")
        for _ in range(18):
            nc.tensor.matmul(
                wps, warm_sb[:, 0:P], warm_sb, start=True, stop=True
            )

        def emit_x_dma(j, only=None):
            """Prefetch chunk j of xq/xk/xv; returns name->tile."""
            tiles = {}
            for name, src in (("q", xq), ("k", xk), ("v", xv)):
                if only and name not in only:
                    continue
                xt = xp.tile([P, ET, QCH], F16, tag="xt", name=f"xt{name}{j}")
                nc.sync.dma_start(out=xt, in_=src[j])
                tiles[name] = xt
            return tiles

        # DMA issue order = first-use order (transfers drain the shared
        # engine pool roughly in issue order, and each dma_start costs
        # ~0.6us of Sync issue time): Q weights + chunk-0 xq gate the
        # first projection matmuls; masks/wo are needed much later.
        wq_sb = const.tile([P, ET, F], F16)
        nc.sync.dma_start(out=wq_sb, in_=wq[:, :, :])
        bq_sb = const.tile([P, F // P], FP32)
        nc.sync.dma_start(out=bq_sb, in_=bq2[:, :])
        xtiles0 = emit_x_dma(0, only="q")
        wk_sb = const.tile([P, ET, F], F16)
        nc.sync.dma_start(out=wk_sb, in_=wk[:, :, :])
        bk_sb = const.tile([P, F // P], FP32)
        nc.sync.dma_start(out=bk_sb, in_=bk2[:, :])
        xtiles0.update(emit_x_dma(0, only="k"))
        wv_sb = const.tile([P, ET, F], F16)
        nc.sync.dma_start(out=wv_sb, in_=wv[:, :, :])
        bvb_sb = const.tile([P, F], FP32)
        nc.sync.dma_start(out=bvb_sb, in_=bvb[:, :])
        xtiles0.update(emit_x_dma(0, only="v"))
        msk_sb = const.tile([P, KPQ, QCH], F16)
        nc.sync.dma_start(out=msk_sb, in_=msk[:, :, :])
        xtiles_next = emit_x_dma(1) if NQ > 1 else None
        wo_sb = const.tile([P, F // P, E], F16)
        nc.sync.dma_start(out=wo_sb, in_=wo[:, :, :])
        ones_f32 = const.tile([P, D], FP32)
        nc.vector.memset(ones_f32, 1.0)

        qT_sb = const.tile([P, F // P, S], F16)
        kT_sb = const.tile([P, F // P, S], F16)
        # V with a trailing ones column: AV matmul emits the softmax
        # denominator as PSUM row D for free.
        vo_sb = const.tile([P, KT, HL, D + 1], F16)
        nc.scalar.activation(
            vo_sb[:, :, :, D:D + 1],
            ones_f32[:, 0:KT * HL].rearrange("p (a b c) -> p a b c", a=KT, b=HL, c=1),
            AF.Copy,
        )
        oT_sb = const.tile([P, F // P, S], F16)

        # ---- emission helpers ----
        def proj_groups(j, xtiles):
            """8 closures: Q blk0/1, K blk0/1, V sl0..3 for chunk j."""
            groups = []

            def qk_group(xt, w_sb, b_sb, dst, blk):
                def emit():
                    acc = pp.tile([P, QCH], FP32, tag="acc")
                    for et in range(ET):
                        nc.tensor.matmul(
                            acc,
                            w_sb[:, et, ds(blk * P, P)],
                            xt[:, et, :],
                            start=(et == 0),
                            stop=(et == ET - 1),
                        )
                    nc.vector.tensor_scalar_add(
                        dst[:, blk, ds(j * QCH, QCH)], acc, b_sb[:, blk:blk + 1]
                    )
                return emit

            def v_group(xt, sl):
                def emit():
                    st = j * (QCH // P) + sl
                    acc = pp.tile([P, QCH], FP32, tag="acc")
                    for et in range(ET):
                        nc.tensor.matmul(
                            acc[:, 0:F],
                            xt[:, et, ds(sl * P, P)],
                            wv_sb[:, et, :],
                            start=(et == 0),
                            stop=(et == ET - 1),
                        )
                    for h in range(HL):
                        nc.vector.tensor_add(
                            vo_sb[:, st, h, 0:D],
                            acc[:, ds(h * D, D)],
                            bvb_sb[:, ds(h * D, D)],
                        )
                return emit

            for blk in range(F // P):
                groups.append(qk_group(xtiles["q"], wq_sb, bq_sb, qT_sb, blk))
            for blk in range(F // P):
                groups.append(qk_group(xtiles["k"], wk_sb, bk_sb, kT_sb, blk))
            for sl in range(QCH // P):
                groups.append(v_group(xtiles["v"], sl))
            return groups

        def outproj_groups(j, tail=False):
            """8 closures: one per output row-block eb for chunk j.  In the
            tail (last chunk, nothing left to overlap) the PSUM->SBUF casts
            alternate between DVE and ACT so they drain ~2x faster."""
            groups = []

            def eb_group(eb):
                def emit():
                    acc = pp.tile([P, QCH], FP32, tag="acc")
                    for fb in range(F // P):
                        nc.tensor.matmul(
                            acc,
                            wo_sb[:, fb, ds(eb * P, P)],
                            oT_sb[:, fb, ds(j * QCH, QCH)],
                            start=(fb == 0),
                            stop=(fb == F // P - 1),
                        )
                    ot = opool.tile([P, QCH], F16, tag="ot")
                    if tail and eb % 2 == 1:
                        nc.scalar.activation(ot, acc, AF.Copy)
                    else:
                        nc.vector.tensor_copy(ot, acc)
                    nc.sync.dma_start(
                        out=outT[ds(eb * P, P), ds(j * QCH, QCH)], in_=ot
                    )
                return emit

            for eb in range(E // P):
                groups.append(eb_group(eb))
            return groups

        def emit_normalize(j, hp, po_pair, tail=False):
            """oT[h, jQ] = po/den for both heads of pair hp of chunk j.
            The chain is deliberately PE-free (DVE recip on the den row,
            GPSIMD partition broadcast, DVE multiply) so the next pair's
            PV matmuls -- which reuse these PSUM banks -- never wait on a
            PE instruction emitted after them (in-order PE queue)."""
            blkh = hp
            for half in range(2):
                po_t = po_pair[half]
                doff = half * D
                rec = prn.tile([P, QCH], FP32, tag="rec")
                den = prn.tile([P, QCH], FP32, tag="den")
                nc.scalar.activation(den[0:1, :], po_t[D:D + 1, :], AF.Copy)
                rr = prn.tile([P, QCH], FP32, tag="rr")
                nc.vector.reciprocal_approx_fast(rr[0:1, :], den[0:1, :])
                nc.gpsimd.partition_broadcast(rec[0:D, :], rr[0:1, :])
                nc.vector.tensor_mul(
                    oT_sb[doff:doff + D, blkh, ds(j * QCH, QCH)],
                    po_t[0:D, :],
                    rec[0:D, :],
                )

        def attention_pair(j, hp, filler, stride=2):
            """Scores+exp+mask+PV for head pair hp (heads 2hp, 2hp+1) of
            chunk j.  The two heads' score matmuls are issued back-to-back
            with lhsT base partitions 0/64 -> concurrent PE row groups.
            Pops filler closures between k-tiles to keep PE dense."""
            blkh = hp
            nkt = KPQ * (j + 1) if causal else KT

            def trunc(kt):
                if causal and kt >= KPQ * j:
                    return (kt - KPQ * j) * P
                return 0

            po_pair = (po.tile([P, QCH], FP32, tag="po", name="poA"),
                       po.tile([P, QCH], FP32, tag="po", name="poB"))
            for kt in range(nkt):
                t0 = trunc(kt)
                w = QCH - t0
                sc = psc.tile([P, 2, QCH], FP32, tag="sc")
                for half in range(2):
                    doff = half * D
                    nc.tensor.matmul(
                        sc[:, half, ds(t0, w)],
                        kT_sb[doff:doff + D, blkh, ds(kt * P, P)],
                        qT_sb[doff:doff + D, blkh, ds(j * QCH + t0, w)],
                        start=True,
                        stop=True,
                    )
                pt = pex.tile([P, 2, QCH], F16, tag="pt")
                nc.scalar.activation(
                    pt[:, :, ds(t0, w)], sc[:, :, ds(t0, w)], AF.Exp
                )
                if causal and kt >= KPQ * j:
                    # NOTE: masking the full [t0, 512) range (not just the
                    # 128-col partial triangle) keeps the mask op the sole
                    # last-writer of the PV read range -- narrowing it to
                    # 128 cols measured 27us SLOWER (extra PE-queue
                    # semaphore wait per diagonal PV).
                    t = kt - KPQ * j
                    for half in range(2):
                        nc.vector.tensor_mul(
                            pt[:, half, ds(t0, w)],
                            pt[:, half, ds(t0, w)],
                            msk_sb[:, t, ds(t0, w)],
                        )
                for half in range(2):
                    h = 2 * hp + half
                    nc.tensor.matmul(
                        po_pair[half][0:D + 1, ds(t0, w)],
                        vo_sb[:, kt, h, :],
                        pt[:, half, ds(t0, w)],
                        start=(kt == 0),
                        stop=(kt == nkt - 1),
                    )
                if filler and kt % stride == stride - 1:
                    filler.popleft()()
            return po_pair

        # ---- pipelined main loop ----
        for g in proj_groups(0, xtiles0):
            g()

        reserve = deque()
        for j in range(NQ):
            filler = deque()
            if j + 1 < NQ:
                filler.extend(proj_groups(j + 1, xtiles_next))
                if j + 2 < NQ:
                    xtiles_next = emit_x_dma(j + 2)
            if j >= 1:
                op_groups = outproj_groups(j - 1)
                if j == NQ - 1:
                    # hold 4 groups back: they fill the PE while the last
                    # pair's normalization chain drains on ACT/DVE/GPSIMD.
                    reserve.extend(op_groups[4:])
                    op_groups = op_groups[:4]
                filler.extend(op_groups)
            for hp in range(HL // 2):
                po_pair = attention_pair(j, hp, filler)
                emit_normalize(j, hp, po_pair)
            while filler:
                filler.popleft()()

        while reserve:
            reserve.popleft()()
        for g in outproj_groups(NQ - 1, tail=True):
            g()

    nc.compile()
    return nc


def make_masks(S=S_FULL):
    KPQ = QCH // P
    m = np.zeros((P, KPQ, QCH), np.float32)
    for t in range(KPQ):
        kk = np.arange(P)[:, None]
        qq = np.arange(QCH)[None, :]
        m[:, t, :] = (qq >= kk + P * t).astype(np.float32)
    return m


def _pack_x(xT, S):
    """[E, S] fp -> chunk-major [NQ, P, ET, QCH] fp16 (contiguous runs)."""
    NQ, ET = S // QCH, E // P
    x = xT.reshape(ET, P, NQ, QCH).transpose(2, 1, 0, 3)
    return np.ascontiguousarray(x.astype(np.float16))


def _pack_w(wT, width):
    """[E, width] -> [P, ET, width] fp16."""
    ET = E // P
    w = wT.reshape(ET, P, width).transpose(1, 0, 2)
    return np.ascontiguousarray(w.astype(np.float16))


def make_in_maps(query, key, value, Wq, bq, Wk, bk, Wv, bv, Wo, bo, S=S_FULL):
    scale = float(D) ** -0.5
    q = np.asarray(query, np.float32)
    k = np.asarray(key, np.float32)
    v = np.asarray(value, np.float32)
    Wq = np.asarray(Wq, np.float32)
    Wk = np.asarray(Wk, np.float32)
    Wv = np.asarray(Wv, np.float32)
    Wo = np.asarray(Wo, np.float32)
    bq = np.asarray(bq, np.float32)
    bk = np.asarray(bk, np.float32)
    bv = np.asarray(bv, np.float32)
    masks = make_masks(S)
    in_maps = []
    for c in range(NCORES):
        b, tp = divmod(c, TP)
        rows = slice(tp * F, (tp + 1) * F)
        wo_p = Wo[:, rows].T.reshape(F // P, P, E).transpose(1, 0, 2)
        in_maps.append({
            "xq": _pack_x(q[b].T, S),
            "xk": _pack_x(k[b].T, S),
            "xv": _pack_x(v[b].T, S),
            "wq": _pack_w((Wq[rows] * scale).T, F),
            "wk": _pack_w(Wk[rows].T, F),
            "wv": _pack_w(Wv[rows].T, F),
            "bq2": np.ascontiguousarray((bq[rows] * scale).reshape(F // P, P).T),
            "bk2": np.ascontiguousarray(bk[rows].reshape(F // P, P).T),
            "bvb": np.ascontiguousarray(np.broadcast_to(bv[rows], (P, F))),
            "wo": np.ascontiguousarray(wo_p.astype(np.float16)),
            "msk": masks.astype(np.float16),
        })
    return in_maps


_CACHE = {}


def _get_nc(causal):
    if causal not in _CACHE:
        _CACHE[causal] = build(S_FULL, causal)
    return _CACHE[causal]


def kernel(query, key, value, Wq, bq, Wk, bk, Wv, bv, Wo, bo, is_causal):
    causal = bool(int(np.asarray(is_causal)))
    nc = _get_nc(causal)
    in_maps = make_in_maps(query, key, value, Wq, bq, Wk, bk, Wv, bv, Wo, bo)
    res = run_bass_kernel_spmd(nc, in_maps, core_ids=list(range(NCORES)))
    out = np.zeros((B, S_FULL, E), np.float32)
    for c in range(NCORES):
        b, tp = divmod(c, TP)
        out[b] += res.results[c]["outT"].astype(np.float32).T
    out += np.asarray(bo, np.float32)
    return out
